# revision 1
# baseline (speedup 1.0000x reference)
"""Trainium2 Bass kernel: transformer encoder layer (S=4096,B=2,D=512,H=8,F=2048),
causal attention + RoPE, distributed over 8 NeuronCores.

Sharding (SPMD: one program, per-core data):
  - LN1+RoPE: sequence-parallel (core c owns s in [512c, 512(c+1)), both batches)
  - AllGather(xr^T, xnorm^T)  [4.2MB/rank]
  - QKV + causal attention: head-parallel (core c owns head c, full S, both b)
  - AllToAll(attn_head^T)     [2MB/rank] -> each core gets all heads for its tokens
  - out_proj + residual + LN2 + FFN: token-parallel (core c owns its s-slice)
LayerNorm affine params are folded into downstream weights host-side.
Softmax denominators come free from a ones-column appended to V.
"""
import numpy as np
import ml_dtypes
from contextlib import ExitStack

import concourse.bass as bass
import concourse.tile as tile
from concourse import bacc, mybir
from concourse.bass_utils import run_bass_kernel_spmd
from concourse.masks import make_identity

F32 = mybir.dt.float32
F32R = mybir.dt.float32r
BF16 = mybir.dt.bfloat16
AF = mybir.ActivationFunctionType
ALU = mybir.AluOpType

S, B, D, H, Dh, F = 4096, 2, 512, 8, 64, 2048
W = 8                    # cores
SL = S // W              # 512 s-positions per core
TL = SL * B              # 1024 local tokens
EPS = 1e-5
SCALE = 1.0 / float(np.sqrt(Dh))  # 0.125

NT = TL // 128           # 8 local token tiles
NK = D // 128            # 4 contraction chunks over D
NF = F // 128            # 16 chunks over F
NS = S // 128            # 32 key tiles per batch

_NC_CACHE = {}
_PHASE_MARKS = []
_GELU_OVERRIDE = None  # set to AF.Identity in sim tests (CoreSim lacks Gelu)


def _mmr(nc, out, lhsT, rhs, **kw):
    """Matmul with fp32->fp22-reduced operand reads (4x faster PE rows)."""
    nc.tensor.matmul(out, lhsT.bitcast(F32R), rhs.bitcast(F32R), **kw)


def _layer_norm_stats(nc, pool, x_t, eps_sb):
    """Returns (rstd [128,1], negmean_rstd [128,1]) for rows of x_t."""
    stats = pool.tile([128, 6], F32, tag="st")
    nc.vector.bn_stats(out=stats, in_=x_t)
    mv = pool.tile([128, 2], F32, tag="mv")
    nc.vector.bn_aggr(out=mv, in_=stats)
    sd = pool.tile([128, 1], F32, tag="sd")
    nc.scalar.activation(out=sd, in_=mv[:, 1:2], func=AF.Sqrt, bias=eps_sb)
    rstd = pool.tile([128, 1], F32, tag="rs")
    nc.vector.reciprocal(out=rstd, in_=sd)
    nm = pool.tile([128, 1], F32, tag="nm")
    nc.vector.tensor_mul(nm, mv[:, 0:1], rstd)
    nc.vector.tensor_scalar_mul(nm, nm, -1.0)
    return rstd, nm


def _build_nc(flags, n_reps=1):
    """flags = (has_ropeb, has_bq, has_bk, has_bv, has_bo, has_b2)

    n_reps > 1 builds a timing variant with the body unrolled n_reps times
    (same I/O, idempotent) so device time can be read off the slope.
    """
    import os as _os
    has_ropeb, has_bq, has_bk, has_bv, has_bo, has_b2 = flags
    has_bqk = has_bq or has_bk
    skip_cc = bool(int(_os.environ.get("K_SKIP_CC", "0")))
    max_phase = int(_os.environ.get("K_MAX_PHASE", "7"))
    nc = bacc.Bacc("TRN2", target_bir_lowering=False, debug=False, num_devices=W)

    # ---- I/O ----
    src_loc = nc.dram_tensor("src_loc", [TL, D], F32, kind="ExternalInput")
    cosw = nc.dram_tensor("cosw", [SL, D], F32, kind="ExternalInput")
    rotw = nc.dram_tensor("rotw", [SL, D], F32, kind="ExternalInput")
    ropeb = nc.dram_tensor("ropeb", [SL, D], F32, kind="ExternalInput") if has_ropeb else None
    wqk_t = nc.dram_tensor("wqk_t", [D, 2 * Dh], BF16, kind="ExternalInput")
    wv_t = nc.dram_tensor("wv_t", [D, Dh], BF16, kind="ExternalInput")
    bqk = nc.dram_tensor("bqk", [2 * Dh], F32, kind="ExternalInput")
    bv = nc.dram_tensor("bv", [Dh], F32, kind="ExternalInput")
    wo_t = nc.dram_tensor("wo_t", [D, D], BF16, kind="ExternalInput")
    bo = nc.dram_tensor("bo", [D], F32, kind="ExternalInput")
    w1_t = nc.dram_tensor("w1_t", [D, F], F32R, kind="ExternalInput")
    b1p = nc.dram_tensor("b1p", [F], F32, kind="ExternalInput")
    w2_t = nc.dram_tensor("w2_t", [F, D], F32R, kind="ExternalInput")
    b2 = nc.dram_tensor("b2", [D], F32, kind="ExternalInput")
    out_loc = nc.dram_tensor("out_loc", [TL, D], F32, kind="ExternalOutput")

    with tile.TileContext(nc) as tc:
      for _rep in range(n_reps):
       with ExitStack() as top:
        dram = top.enter_context(tc.tile_pool(name="dram", bufs=1, space="DRAM"))
        consts = top.enter_context(tc.tile_pool(name="consts", bufs=1))
        persist = top.enter_context(tc.tile_pool(name="persist", bufs=1))

        # ---------- constants ----------
        ident = consts.tile([128, 128], F32)
        make_identity(nc, ident)
        identb = consts.tile([128, 128], BF16)
        make_identity(nc, identb)
        # causal diag masks: masks[:, j, q] = 1.0 if q >= k + j*128 else 0.0
        masks = consts.tile([128, 4, 512], BF16)
        for j in range(4):
            nc.gpsimd.memset(masks[:, j, :], 1.0)
            nc.gpsimd.affine_select(
                out=masks[:, j, :], in_=masks[:, j, :],
                compare_op=ALU.is_ge, fill=0.0,
                base=-j * 128, channel_multiplier=-1, pattern=[[1, 512]],
            )
        eps_sb = consts.tile([128, 1], F32)
        nc.vector.memset(eps_sb, EPS)
        bq_sb = consts.tile([Dh, 1], F32)
        nc.sync.dma_start(out=bq_sb, in_=bqk[0:Dh, None])
        bk_sb = consts.tile([Dh, 1], F32)
        nc.sync.dma_start(out=bk_sb, in_=bqk[Dh:2 * Dh, None])
        bv_sb = consts.tile([Dh, 1], F32)
        nc.sync.dma_start(out=bv_sb, in_=bv[:, None])
        bo_bc = consts.tile([128, D], F32)
        if has_bo:
            bo_row = consts.tile([1, D], F32)
            nc.sync.dma_start(out=bo_row, in_=bo[None, :])
            nc.gpsimd.partition_broadcast(bo_bc, bo_row)
        b2_bc = consts.tile([128, D], F32)
        if has_b2:
            b2_row = consts.tile([1, D], F32)
            nc.sync.dma_start(out=b2_row, in_=b2[None, :])
            nc.gpsimd.partition_broadcast(b2_bc, b2_row)
        b1_sb = consts.tile([128, NF], F32)
        nc.sync.dma_start(out=b1_sb, in_=b1p.rearrange("(m p) -> p m", p=128))

        # collective buffers
        cc1a_in = dram.tile([NK, 128, TL], BF16)
        cc1a_out = dram.tile([W, NK, 128, TL], BF16, addr_space="Shared")
        cc1b_in = dram.tile([NK, 128, TL], BF16)
        cc1b_out = dram.tile([W, NK, 128, TL], BF16, addr_space="Shared")
        cc2_in = [dram.tile([W, Dh, SL], BF16, name=f"cc2_in_{b}") for b in range(B)]
        cc2_out = [dram.tile([W, Dh, SL], BF16, name=f"cc2_out_{b}") for b in range(B)]


        _PHASE_MARKS.append(("consts", nc.next_id()))
        # ================= P1: LN1 + RoPE + transpose =================
        with ExitStack() as ctx:
            sb = ctx.enter_context(tc.tile_pool(name="p1", bufs=3))
            small = ctx.enter_context(tc.tile_pool(name="p1s", bufs=4))
            trps = ctx.enter_context(tc.tile_pool(name="p1ps", bufs=4, space="PSUM"))
            for t in range(NT):
                s_t = sb.tile([128, D], F32, tag="s")
                nc.sync.dma_start(out=s_t, in_=src_loc[t * 128:(t + 1) * 128, :])
                rstd, nm = _layer_norm_stats(nc, small, s_t, eps_sb)
                xn_t = sb.tile([128, D], F32, tag="xn")
                nc.vector.tensor_scalar(
                    out=xn_t, in0=s_t, scalar1=rstd, scalar2=nm,
                    op0=ALU.mult, op1=ALU.add,
                )
                # RoPE (ln1 affine folded into cosw/rotw/ropeb host-side)
                sc = t % (SL // 128)
                cosw_t = sb.tile([128, D], F32, tag="cw")
                nc.sync.dma_start(out=cosw_t, in_=cosw[sc * 128:(sc + 1) * 128, :])
                rotw_t = sb.tile([128, D], F32, tag="rw")
                nc.sync.dma_start(out=rotw_t, in_=rotw[sc * 128:(sc + 1) * 128, :])
                xr_t = sb.tile([128, D], F32, tag="xr")
                rt = sb.tile([128, D], F32, tag="rt")
                xnv = xn_t.rearrange("p (h i two) -> p h i two", h=H, two=2)
                rtv = rt.rearrange("p (h d) -> p h d", h=H)
                rwv = rotw_t.rearrange("p (h d) -> p h d", h=H)
                # rt[:, :, :32] = xn[:, :, 1::2] * rotw[:, :, :32]
                nc.vector.tensor_mul(rtv[:, :, 0:32], xnv[:, :, :, 1], rwv[:, :, 0:32])
                # rt[:, :, 32:] = xn[:, :, 0::2] * rotw[:, :, 32:]
                nc.vector.tensor_mul(rtv[:, :, 32:64], xnv[:, :, :, 0], rwv[:, :, 32:64])
                nc.vector.tensor_mul(xr_t, xn_t, cosw_t)
                nc.vector.tensor_add(xr_t, xr_t, rt)
                if has_ropeb:
                    rb_t = sb.tile([128, D], F32, tag="rb")
                    nc.sync.dma_start(out=rb_t, in_=ropeb[sc * 128:(sc + 1) * 128, :])
                    nc.vector.tensor_add(xr_t, xr_t, rb_t)
                # transpose to D-major; xr ships to AG-a, xn to AG-b
                for cc_dst, src_tile in ((cc1a_in, xr_t), (cc1b_in, xn_t)):
                    ps = trps.tile([128, 512], F32, tag="tr")
                    for k in range(NK):
                        nc.tensor.transpose(ps[:, k * 128:(k + 1) * 128],
                                            src_tile[:, k * 128:(k + 1) * 128], ident)
                    tmp = sb.tile([128, NK, 128], BF16, tag="tmp")
                    nc.vector.tensor_copy(tmp, ps.rearrange("p (k i) -> p k i", k=NK))
                    nc.sync.dma_start(
                        out=cc_dst[:, :, t * 128:(t + 1) * 128].rearrange(
                            "k p i -> p k i"),
                        in_=tmp)

        _PHASE_MARKS.append(("P1", nc.next_id()))
        if not skip_cc and max_phase >= 2:
            nc.gpsimd.collective_compute(
                "AllGather", ALU.bypass,
                ins=[cc1a_in.opt()], outs=[cc1a_out.opt()],
                replica_groups=[list(range(W))],
            )
        if not skip_cc and max_phase >= 2:
            nc.gpsimd.collective_compute(
                "AllGather", ALU.bypass,
                ins=[cc1b_in.opt()], outs=[cc1b_out.opt()],
                replica_groups=[list(range(W))],
            )

        with ExitStack() as actx:
          if max_phase >= 3:
            act = actx.enter_context(tc.tile_pool(name="act", bufs=1))
            qT = act.tile([Dh, B, S], BF16)
            kT = act.tile([Dh, B, S], BF16)
            vS = act.tile([128, B, NS, 65], BF16)
            nc.vector.memset(vS[:, :, :, 64:65], 1.0)
            attnT = act.tile([Dh, B, S], BF16)

            _PHASE_MARKS.append(("AGs", nc.next_id()))
            # ============ P2: QKV projections (head h = this core) ============
            with ExitStack() as ctx:
                sb = ctx.enter_context(tc.tile_pool(name="p2", bufs=2))
                wpool = ctx.enter_context(tc.tile_pool(name="p2w", bufs=1))
                qkps = ctx.enter_context(tc.tile_pool(name="p2ps", bufs=2, space="PSUM"))
                vtps = ctx.enter_context(tc.tile_pool(name="p2vt", bufs=2, space="PSUM"))
                wqk_sb = wpool.tile([128, NK, 2 * Dh], BF16)
                nc.sync.dma_start(out=wqk_sb, in_=wqk_t.rearrange("(k p) m -> p k m", p=128))
                wv_sb = wpool.tile([128, NK, Dh], BF16)
                nc.sync.dma_start(out=wv_sb, in_=wv_t.rearrange("(k p) m -> p k m", p=128))
                for j in range(W):
                    for b in range(B):
                        tok0 = b * SL
                        scol = j * SL
                        xr_in = sb.tile([128, NK, SL], BF16, tag="xrin")
                        nc.sync.dma_start(
                            out=xr_in,
                            in_=cc1a_out[j].rearrange("k p t -> p k t")[:, :, tok0:tok0 + SL])
                        for (lo, dstT, bias_sb, hasb) in (
                                (0, qT, bq_sb, has_bq), (Dh, kT, bk_sb, has_bk)):
                            ps = qkps.tile([Dh, SL], F32, tag="qk")
                            for k in range(NK):
                                nc.tensor.matmul(ps, wqk_sb[:, k, lo:lo + Dh],
                                                 xr_in[:, k, :],
                                                 start=(k == 0), stop=(k == NK - 1))
                            if hasb:
                                nc.vector.tensor_scalar_add(
                                    dstT[:, b, scol:scol + SL], ps, bias_sb)
                            else:
                                nc.vector.tensor_copy(dstT[:, b, scol:scol + SL], ps)
                for j in range(W):
                    for b in range(B):
                        tok0 = b * SL
                        scol = j * SL
                        xn_in = sb.tile([128, NK, SL], BF16, tag="xnin")
                        nc.sync.dma_start(
                            out=xn_in,
                            in_=cc1b_out[j].rearrange("k p t -> p k t")[:, :, tok0:tok0 + SL])
                        # v -> token-major via PE transpose
                        ps = qkps.tile([Dh, SL], F32, tag="v")
                        for k in range(NK):
                            nc.tensor.matmul(ps, wv_sb[:, k, :], xn_in[:, k, :],
                                             start=(k == 0), stop=(k == NK - 1))
                        vtmp = sb.tile([Dh, SL], BF16, tag="vtmp")
                        if has_bv:
                            nc.vector.tensor_scalar_add(vtmp, ps, bv_sb)
                        else:
                            nc.vector.tensor_copy(vtmp, ps)
                        vt = vtps.tile([128, 4, Dh], BF16, tag="vt")
                        for q4 in range(4):
                            nc.tensor.transpose(vt[:, q4, :],
                                                vtmp[:, q4 * 128:(q4 + 1) * 128],
                                                identb[0:Dh, 0:Dh])
                        nc.vector.tensor_copy(vS[:, b, j * 4:(j + 1) * 4, 0:64], vt)

            _PHASE_MARKS.append(("P2", nc.next_id()))
            # ============ P4: causal attention (software-pipelined) ============
            if max_phase >= 4:
              with ExitStack() as ctx:
                  expp = ctx.enter_context(tc.tile_pool(name="p4e", bufs=4))
                  nrm = ctx.enter_context(tc.tile_pool(name="p4n", bufs=3))
                  scps = ctx.enter_context(tc.tile_pool(name="p4s", bufs=2, space="PSUM"))
                  atps = ctx.enter_context(tc.tile_pool(name="p4a", bufs=2, space="PSUM"))
                  # flat job list: (b, qb, pair)
                  jobs = [(b, qb, p)
                          for b in range(B) for qb in range(8)
                          for p in range(2 * (qb + 1))]
                  sc_ps = {}
                  pa_cur = {}

                  def emit_sc(job):
                      b, qb, p = job
                      q_rhs = qT[:, b, qb * 512:(qb + 1) * 512]
                      ps = scps.tile([128, 1024], F32, tag="sc", name="sc_ps_t")
                      for i in range(2):
                          kt = p * 2 + i
                          nc.tensor.matmul(ps[:, i * 512:(i + 1) * 512],
                                           kT[:, b, kt * 128:(kt + 1) * 128],
                                           q_rhs, start=True, stop=True)
                      sc_ps[job] = ps

                  def emit_pv(job):
                      b, qb, p = job
                      nkt = 4 * (qb + 1)
                      ps = sc_ps.pop(job)
                      if p == 0:
                          pa_cur[(b, qb)] = atps.tile([65, 512], F32, tag="pa",
                                                      name="pa_t")
                      pa = pa_cur[(b, qb)]
                      ex = expp.tile([128, 1024], BF16, tag="ex", name="ex_t")
                      nc.scalar.activation(out=ex, in_=ps, func=AF.Exp, scale=SCALE)
                      for i in range(2):
                          kt = p * 2 + i
                          jm = kt - (nkt - 4)
                          if jm >= 0:
                              nc.vector.tensor_mul(ex[:, i * 512:(i + 1) * 512],
                                                   ex[:, i * 512:(i + 1) * 512],
                                                   masks[:, jm, :])
                          nc.tensor.matmul(pa, vS[:, b, kt, :],
                                           ex[:, i * 512:(i + 1) * 512],
                                           start=(kt == 0), stop=(kt == nkt - 1))
                      if p == 2 * (qb + 1) - 1:
                          # normalization tail for this (b, qb)
                          pa = pa_cur.pop((b, qb))
                          pa_sb = nrm.tile([65, 512], F32, tag="pasb")
                          nc.vector.tensor_copy(pa_sb, pa)
                          sums = nrm.tile([1, 512], F32, tag="sums")
                          nc.sync.dma_start(out=sums, in_=pa_sb[64:65, :])
                          rcp = nrm.tile([1, 512], F32, tag="rcp")
                          nc.vector.reciprocal(rcp, sums)
                          rcp_bc = nrm.tile([Dh, 512], F32, tag="rbc")
                          nc.gpsimd.partition_broadcast(rcp_bc, rcp)
                          nc.vector.tensor_mul(
                              attnT[:, b, qb * 512:(qb + 1) * 512],
                              pa_sb[0:64, :], rcp_bc)

                  emit_sc(jobs[0])
                  for idx, job in enumerate(jobs):
                      if idx + 1 < len(jobs):
                          emit_sc(jobs[idx + 1])
                      emit_pv(job)
                      # ship + exchange each batch as soon as it completes
                      b, qb, p = job
                      if qb == 7 and p == 2 * (qb + 1) - 1:
                          nc.sync.dma_start(
                              out=cc2_in[b].rearrange("j d i -> d j i"),
                              in_=attnT[:, b, :].rearrange("d (j i) -> d j i", j=W))
                          if not skip_cc and max_phase >= 5:
                              nc.gpsimd.collective_compute(
                                  "AllToAll", ALU.bypass,
                                  ins=[cc2_in[b].opt()], outs=[cc2_out[b].opt()],
                                  replica_groups=[list(range(W))],
                              )

        _PHASE_MARKS.append(("P4", nc.next_id()))
        # ========== P5/P6 ==========
        late_ctx = top.enter_context(ExitStack())
        late = late_ctx.enter_context(tc.tile_pool(name="late", bufs=1))
        out1 = late.tile([128, NT, D], F32)   # post-attention residual stream
        yT = late.tile([128, NK, TL], F32R)    # LN2 output, D-major

        # ========== P5: out_proj + residual + LN2 (+ transpose y) ==========
        if max_phase >= 6:
          with ExitStack() as ctx:
              sb = ctx.enter_context(tc.tile_pool(name="p5", bufs=3))
              small = ctx.enter_context(tc.tile_pool(name="p5s", bufs=4))
              wpool = ctx.enter_context(tc.tile_pool(name="p5w", bufs=1))
              ops = ctx.enter_context(tc.tile_pool(name="p5ps", bufs=2, space="PSUM"))
              trps = ctx.enter_context(tc.tile_pool(name="p5tr", bufs=2, space="PSUM"))
              wo_sb = wpool.tile([128, NK, D], BF16)
              nc.sync.dma_start(out=wo_sb, in_=wo_t.rearrange("(k p) n -> p k n", p=128))
              for t in range(NT):
                  b, sc = t // (NT // B), t % (NT // B)
                  po = ops.tile([128, D], F32, tag="po")
                  for k in range(NK):
                      a_sb = sb.tile([128, 128], BF16, tag="a")
                      nc.sync.dma_start(
                          out=a_sb,
                          in_=cc2_out[b][2 * k:2 * k + 2, :,
                                         sc * 128:(sc + 1) * 128].rearrange(
                                             "e d i -> (e d) i"))
                      nc.tensor.matmul(po, a_sb, wo_sb[:, k, :],
                                       start=(k == 0), stop=(k == NK - 1))
                  s_t = sb.tile([128, D], F32, tag="s")
                  nc.sync.dma_start(out=s_t, in_=src_loc[t * 128:(t + 1) * 128, :])
                  o1 = out1[:, t, :]
                  nc.vector.tensor_add(o1, po, s_t)
                  if has_bo:
                      nc.vector.tensor_add(o1, o1, bo_bc)
                  # LN2 (affine folded into w1_t/b1p host-side)
                  rstd, nm = _layer_norm_stats(nc, small, o1, eps_sb)
                  y_t = sb.tile([128, D], F32, tag="y")
                  nc.vector.tensor_scalar(out=y_t, in0=o1, scalar1=rstd, scalar2=nm,
                                          op0=ALU.mult, op1=ALU.add)
                  ps = trps.tile([128, 512], F32, tag="tr")
                  for k in range(NK):
                      nc.tensor.transpose(ps[:, k * 128:(k + 1) * 128],
                                          y_t[:, k * 128:(k + 1) * 128], ident)
                  nc.vector.tensor_copy(
                      yT[:, :, t * 128:(t + 1) * 128],
                      ps.rearrange("p (k i) -> p k i", k=NK))

        _PHASE_MARKS.append(("P5", nc.next_id()))
        # ================= P6: FFN + final residual =================
        if max_phase >= 7:
          with ExitStack() as ctx:
              sb = ctx.enter_context(tc.tile_pool(name="p6", bufs=3))
              wpool = ctx.enter_context(tc.tile_pool(name="p6w", bufs=1))
              hps = ctx.enter_context(tc.tile_pool(name="p6h", bufs=2, space="PSUM"))
              o2ps = ctx.enter_context(tc.tile_pool(name="p6o", bufs=1, space="PSUM"))
              w1_sb = wpool.tile([128, NK, F], F32R)
              nc.sync.dma_start(out=w1_sb, in_=w1_t.rearrange("(k p) n -> p k n", p=128))
              w2_sb = wpool.tile([128, NF, D], F32R)
              nc.sync.dma_start(out=w2_sb, in_=w2_t.rearrange("(m p) n -> p m n", p=128))
              for th in range(2):
                  po2 = [o2ps.tile([128, D], F32, tag=f"po2_{tq}", name=f"po2_{tq}")
                         for tq in range(4)]
                  for m in range(NF):
                      ph = hps.tile([128, 512], F32, tag="ph")
                      for k in range(NK):
                          nc.tensor.matmul(ph, w1_sb[:, k, m * 128:(m + 1) * 128],
                                           yT[:, k, th * 512:(th + 1) * 512],
                                           start=(k == 0), stop=(k == NK - 1))
                      hT = sb.tile([128, 512], F32R, tag="hT")
                      nc.scalar.activation(out=hT, in_=ph,
                                           func=_GELU_OVERRIDE or AF.Gelu,
                                           bias=b1_sb[:, m:m + 1])
                      for tq in range(4):
                          nc.tensor.matmul(po2[tq], hT[:, tq * 128:(tq + 1) * 128],
                                           w2_sb[:, m, :],
                                           start=(m == 0), stop=(m == NF - 1))
                  for tq in range(4):
                      t = th * 4 + tq
                      fin = sb.tile([128, D], F32, tag="fin")
                      nc.vector.tensor_add(fin, po2[tq], out1[:, t, :])
                      if has_b2:
                          nc.vector.tensor_add(fin, fin, b2_bc)
                      nc.sync.dma_start(out=out_loc[t * 128:(t + 1) * 128, :], in_=fin)

        _PHASE_MARKS.append(("P6", nc.next_id()))
        if max_phase < 7:
            with tc.tile_pool(name="dummy", bufs=1) as dp:
                dt_ = dp.tile([128, D], F32)
                nc.vector.memset(dt_, 0.0)
                for i in range(TL // 128):
                    nc.sync.dma_start(out=out_loc[i * 128:(i + 1) * 128, :], in_=dt_)
    nc.compile()
    return nc


def _prep(inputs):
    src = np.asarray(inputs["src"], np.float32)
    cos = np.asarray(inputs["rotary_cos"], np.float32).reshape(S, Dh)
    sin = np.asarray(inputs["rotary_sin"], np.float32).reshape(S, Dh)
    ipw = np.asarray(inputs["in_proj_w"], np.float32)
    ipb = np.asarray(inputs["in_proj_b"], np.float32)
    opw = np.asarray(inputs["out_proj_w"], np.float32)
    opb = np.asarray(inputs["out_proj_b"], np.float32)
    w1 = np.asarray(inputs["w1"], np.float32)
    b1 = np.asarray(inputs["b1"], np.float32)
    w2 = np.asarray(inputs["w2"], np.float32)
    b2 = np.asarray(inputs["b2"], np.float32)
    ln1_w = np.asarray(inputs["ln1_w"], np.float32)
    ln1_b = np.asarray(inputs["ln1_b"], np.float32)
    ln2_w = np.asarray(inputs["ln2_w"], np.float32)
    ln2_b = np.asarray(inputs["ln2_b"], np.float32)

    cos_full = np.tile(cos, (1, H))            # [S, D]
    sin_full = np.tile(sin, (1, H))
    d = np.arange(D)
    jj = d % Dh
    hb = d - jj
    src2 = np.where(jj < 32, hb + 2 * jj + 1, hb + 2 * (jj - 32))
    sign = np.where(jj < 32, -1.0, 1.0).astype(np.float32)
    cosw_full = ln1_w[None, :] * cos_full
    rotw_full = (sign[None, :] * ln1_w[src2][None, :]) * sin_full
    ropeb_full = (ln1_b[None, :] * cos_full
                  + (sign[None, :] * ln1_b[src2][None, :]) * sin_full)

    wq, wk, wv = ipw[0:D], ipw[D:2 * D], ipw[2 * D:3 * D]
    bq, bk, bv = ipb[0:D], ipb[D:2 * D], ipb[2 * D:3 * D]
    w1_t = np.ascontiguousarray(ln2_w[:, None] * w1.T, np.float32)   # [D, F]
    b1p = np.ascontiguousarray(ln2_b @ w1.T + b1, np.float32)
    wo_t = np.ascontiguousarray(opw.T)

    flags = (
        bool(np.any(ropeb_full)), bool(np.any(bq)), bool(np.any(bk)),
        bool(np.any(bv) or np.any(ln1_b)), bool(np.any(opb)), bool(np.any(b2)),
    )

    in_maps = []
    for c in range(W):
        h0 = c * Dh
        wv_h = wv[h0:h0 + Dh]                                        # [64, D]
        wv_t_c = np.ascontiguousarray(ln1_w[:, None] * wv_h.T, np.float32)
        bv_c = np.ascontiguousarray(ln1_b @ wv_h.T + bv[h0:h0 + Dh], np.float32)
        m = {
            "src_loc": np.ascontiguousarray(
                src[SL * c:SL * (c + 1)].transpose(1, 0, 2).reshape(TL, D)),
            "cosw": np.ascontiguousarray(cosw_full[SL * c:SL * (c + 1)]),
            "rotw": np.ascontiguousarray(rotw_full[SL * c:SL * (c + 1)]),
            "wqk_t": np.ascontiguousarray(
                np.concatenate([wq[h0:h0 + Dh].T, wk[h0:h0 + Dh].T],
                               axis=1)).astype(ml_dtypes.bfloat16),
            "wv_t": wv_t_c.astype(ml_dtypes.bfloat16),
            "bqk": np.concatenate([bq[h0:h0 + Dh], bk[h0:h0 + Dh]]),
            "bv": bv_c,
            "wo_t": wo_t.astype(ml_dtypes.bfloat16),
            "bo": opb,
            "w1_t": w1_t,
            "b1p": b1p,
            "w2_t": np.ascontiguousarray(w2.T),
            "b2": b2,
        }
        if flags[0]:
            m["ropeb"] = np.ascontiguousarray(ropeb_full[SL * c:SL * (c + 1)])
        in_maps.append(m)
    return in_maps, flags


def _get_nc(flags):
    if flags not in _NC_CACHE:
        _NC_CACHE[flags] = _build_nc(flags)
    return _NC_CACHE[flags]


def kernel(**inputs):
    in_maps, flags = _prep(inputs)
    nc = _get_nc(flags)
    res = run_bass_kernel_spmd(nc, in_maps, core_ids=list(range(W)))
    out = np.empty((S, B, D), np.float32)
    for c in range(W):
        ol = res.results[c]["out_loc"].reshape(B, SL, D)
        out[SL * c:SL * (c + 1)] = ol.transpose(1, 0, 2)
    return out



# revision 9
# speedup vs baseline: 1.1865x; 1.1865x over previous
"""Trainium2 Bass kernel: transformer encoder layer (S=4096,B=2,D=512,H=8,F=2048),
causal attention + RoPE, distributed over 8 NeuronCores.

Sharding (SPMD: one program, per-core data):
  - LN1+RoPE: sequence-parallel (core c owns s in [512c, 512(c+1)), both batches)
  - QKV projections: token-parallel (each core projects q,k,v of ALL heads for
    its own tokens, emitting q/k head-major and v token-major directly)
  - AllToAll(qkv)             [384KB/rank] -> core c gets head c for all tokens
  - causal attention: head-parallel (core c owns head c, full S, both b)
  - AllToAll(attn_head^T)     [512KB/rank, per batch] -> all heads, own tokens
  - out_proj + residual + LN2 + FFN: token-parallel (core c owns its s-slice)
LayerNorm affine params are folded into downstream weights host-side.
Softmax denominators come free from a ones-column appended to V.
All weights preloaded to SBUF up front so their DMA overlaps early compute.
"""
import numpy as np
import ml_dtypes
from contextlib import ExitStack

import concourse.bass as bass
import concourse.tile as tile
from concourse import bacc, mybir
from concourse.bass_utils import run_bass_kernel_spmd
from concourse.masks import make_identity

F32 = mybir.dt.float32
F32R = mybir.dt.float32r
BF16 = mybir.dt.bfloat16
AF = mybir.ActivationFunctionType
ALU = mybir.AluOpType

S, B, D, H, Dh, F = 4096, 2, 512, 8, 64, 2048
W = 8                    # cores
SL = S // W              # 512 s-positions per core
TL = SL * B              # 1024 local tokens
EPS = 1e-5
SCALE = 1.0 / float(np.sqrt(Dh))  # 0.125

NT = TL // 128           # 8 local token tiles
NK = D // 128            # 4 contraction chunks over D
NF = F // 128            # 16 chunks over F
NS = S // 128            # 32 key tiles per batch
QKC = 2 * Dh * H         # 1024 packed q,k output columns (h-major)
CCW = QKC + NT * Dh      # 1536 columns per A2A block

_NC_CACHE = {}
_GELU_OVERRIDE = None  # set to AF.Identity in sim tests (CoreSim lacks Gelu)


def _layer_norm_stats(nc, pool, x_t, eps_sb):
    """Returns (rstd [128,1], negmean_rstd [128,1]) for rows of x_t."""
    stats = pool.tile([128, 6], F32, tag="st")
    nc.vector.bn_stats(out=stats, in_=x_t)
    mv = pool.tile([128, 2], F32, tag="mv")
    nc.vector.bn_aggr(out=mv, in_=stats)
    sd = pool.tile([128, 1], F32, tag="sd")
    nc.scalar.activation(out=sd, in_=mv[:, 1:2], func=AF.Sqrt, bias=eps_sb)
    rstd = pool.tile([128, 1], F32, tag="rs")
    nc.vector.reciprocal(out=rstd, in_=sd)
    nm = pool.tile([128, 1], F32, tag="nm")
    nc.vector.tensor_mul(nm, mv[:, 0:1], rstd)
    nc.vector.tensor_scalar_mul(nm, nm, -1.0)
    return rstd, nm


def _build_nc(flags, n_reps=1):
    """flags = (has_ropeb, has_bqk, has_bv, has_bo, has_b2)

    n_reps > 1 builds a timing variant with the body unrolled n_reps times
    (same I/O, idempotent) so device time can be read off the slope.
    """
    import os as _os
    has_ropeb, has_bqk, has_bv, has_bo, has_b2 = flags
    skip_cc = bool(int(_os.environ.get("K_SKIP_CC", "0")))
    max_phase = int(_os.environ.get("K_MAX_PHASE", "7"))
    nc = bacc.Bacc("TRN2", target_bir_lowering=False, debug=False, num_devices=W)

    # ---- I/O ----
    src_loc = nc.dram_tensor("src_loc", [TL, D], F32, kind="ExternalInput")
    cosw = nc.dram_tensor("cosw", [SL, D], F32, kind="ExternalInput")
    rotw = nc.dram_tensor("rotw", [SL, D], F32, kind="ExternalInput")
    ropeb = nc.dram_tensor("ropeb", [SL, D], F32, kind="ExternalInput") if has_ropeb else None
    wqk_t = nc.dram_tensor("wqk_t", [D, QKC], BF16, kind="ExternalInput")
    wv_t = nc.dram_tensor("wv_t", [D, D], BF16, kind="ExternalInput")
    bqk = nc.dram_tensor("bqk", [128, H], F32, kind="ExternalInput")
    bv = nc.dram_tensor("bv", [D], F32, kind="ExternalInput")
    wo_t = nc.dram_tensor("wo_t", [D, D], BF16, kind="ExternalInput")
    bo = nc.dram_tensor("bo", [D], F32, kind="ExternalInput")
    w1_t = nc.dram_tensor("w1_t", [D, F], F32R, kind="ExternalInput")
    b1p = nc.dram_tensor("b1p", [F], F32, kind="ExternalInput")
    w2_t = nc.dram_tensor("w2_t", [F, D], F32R, kind="ExternalInput")
    b2 = nc.dram_tensor("b2", [D], F32, kind="ExternalInput")
    out_loc = nc.dram_tensor("out_loc", [TL, D], F32, kind="ExternalOutput")

    with tile.TileContext(nc) as tc:
      for _rep in range(n_reps):
       with ExitStack() as top:
        dram = top.enter_context(tc.tile_pool(name="dram", bufs=1, space="DRAM"))
        consts = top.enter_context(tc.tile_pool(name="consts", bufs=1))
        persist = top.enter_context(tc.tile_pool(name="persist", bufs=1))

        # ---------- constants + all weights up front ----------
        ident = consts.tile([128, 128], F32)
        make_identity(nc, ident)
        # causal diag masks: masks[:, j, q] = 1.0 if q >= k + j*128 else 0.0
        masks = consts.tile([128, 4, 512], BF16)
        for j in range(4):
            nc.gpsimd.memset(masks[:, j, :], 1.0)
            nc.gpsimd.affine_select(
                out=masks[:, j, :], in_=masks[:, j, :],
                compare_op=ALU.is_ge, fill=0.0,
                base=-j * 128, channel_multiplier=-1, pattern=[[1, 512]],
            )
        eps_sb = consts.tile([128, 1], F32)
        nc.vector.memset(eps_sb, EPS)
        wqk_sb = consts.tile([128, NK, QKC], BF16)
        nc.sync.dma_start(out=wqk_sb, in_=wqk_t.rearrange("(k p) m -> p k m", p=128))
        wv_sb = consts.tile([128, NK, D], BF16)
        nc.sync.dma_start(out=wv_sb, in_=wv_t.rearrange("(k p) m -> p k m", p=128))
        wo_sb = consts.tile([128, NK, D], BF16)
        nc.sync.dma_start(out=wo_sb, in_=wo_t.rearrange("(k p) n -> p k n", p=128))
        w1_sb = consts.tile([128, NK, F], F32R)
        nc.sync.dma_start(out=w1_sb, in_=w1_t.rearrange("(k p) n -> p k n", p=128))
        w2_sb = consts.tile([128, NF, D], F32R)
        nc.sync.dma_start(out=w2_sb, in_=w2_t.rearrange("(m p) n -> p m n", p=128))
        b1_sb = consts.tile([128, NF], F32)
        nc.sync.dma_start(out=b1_sb, in_=b1p.rearrange("(m p) -> p m", p=128))
        bqk_sb = consts.tile([128, H], F32)
        if has_bqk:
            nc.sync.dma_start(out=bqk_sb, in_=bqk)
        bv_bc = consts.tile([128, D], F32)
        if has_bv:
            bv_row = consts.tile([1, D], F32)
            nc.sync.dma_start(out=bv_row, in_=bv[None, :])
            nc.gpsimd.partition_broadcast(bv_bc, bv_row)
        bo_bc = consts.tile([128, D], F32)
        if has_bo:
            bo_row = consts.tile([1, D], F32)
            nc.sync.dma_start(out=bo_row, in_=bo[None, :])
            nc.gpsimd.partition_broadcast(bo_bc, bo_row)
        b2_bc = consts.tile([128, D], F32)
        if has_b2:
            b2_row = consts.tile([1, D], F32)
            nc.sync.dma_start(out=b2_row, in_=b2[None, :])
            nc.gpsimd.partition_broadcast(b2_bc, b2_row)

        # collective buffers
        ccq_in = dram.tile([W, 128, CCW], BF16)
        ccq_out = dram.tile([W, 128, CCW], BF16)
        cc2_in = [dram.tile([W, Dh, SL], BF16, name=f"cc2_in_{b}") for b in range(B)]
        cc2_out = [dram.tile([W, Dh, SL], BF16, name=f"cc2_out_{b}") for b in range(B)]

        # D-major LN1/RoPE outputs (persist through P2')
        xrT = persist.tile([128, NK, TL], BF16)
        xnT = persist.tile([128, NK, TL], BF16)

        # ============ P1: LN1 + RoPE + transpose (seq-parallel) ============
        # ============ P2': QKV for own tokens, all heads ============
        with ExitStack() as ctx:
            sb = ctx.enter_context(tc.tile_pool(name="p1", bufs=3))
            small = ctx.enter_context(tc.tile_pool(name="p1s", bufs=4))
            trps = ctx.enter_context(tc.tile_pool(name="p1ps", bufs=2, space="PSUM"))
            qkps = ctx.enter_context(tc.tile_pool(name="p2qk", bufs=2, space="PSUM"))
            vps = ctx.enter_context(tc.tile_pool(name="p2v", bufs=2, space="PSUM"))
            stg = ctx.enter_context(tc.tile_pool(name="p2stg", bufs=3))

            def p1_tile(t):
                s_t = sb.tile([128, D], F32, tag="s")
                nc.sync.dma_start(out=s_t, in_=src_loc[t * 128:(t + 1) * 128, :])
                rstd, nm = _layer_norm_stats(nc, small, s_t, eps_sb)
                xn_t = sb.tile([128, D], F32, tag="xn")
                nc.vector.tensor_scalar(
                    out=xn_t, in0=s_t, scalar1=rstd, scalar2=nm,
                    op0=ALU.mult, op1=ALU.add,
                )
                # RoPE (ln1 affine folded into cosw/rotw/ropeb host-side)
                sc = t % (SL // 128)
                cosw_t = sb.tile([128, D], F32, tag="cw")
                nc.sync.dma_start(out=cosw_t, in_=cosw[sc * 128:(sc + 1) * 128, :])
                rotw_t = sb.tile([128, D], F32, tag="rw")
                nc.sync.dma_start(out=rotw_t, in_=rotw[sc * 128:(sc + 1) * 128, :])
                xr_t = sb.tile([128, D], F32, tag="xr")
                rt = sb.tile([128, D], F32, tag="rt")
                xnv = xn_t.rearrange("p (h i two) -> p h i two", h=H, two=2)
                rtv = rt.rearrange("p (h d) -> p h d", h=H)
                rwv = rotw_t.rearrange("p (h d) -> p h d", h=H)
                # rt[:, :, :32] = xn[:, :, 1::2] * rotw[:, :, :32]
                nc.vector.tensor_mul(rtv[:, :, 0:32], xnv[:, :, :, 1], rwv[:, :, 0:32])
                # rt[:, :, 32:] = xn[:, :, 0::2] * rotw[:, :, 32:]
                nc.vector.tensor_mul(rtv[:, :, 32:64], xnv[:, :, :, 0], rwv[:, :, 32:64])
                nc.vector.tensor_mul(xr_t, xn_t, cosw_t)
                nc.vector.tensor_add(xr_t, xr_t, rt)
                if has_ropeb:
                    rb_t = sb.tile([128, D], F32, tag="rb")
                    nc.sync.dma_start(out=rb_t, in_=ropeb[sc * 128:(sc + 1) * 128, :])
                    nc.vector.tensor_add(xr_t, xr_t, rb_t)
                # transpose both to D-major; xr -> DVE copy, xn -> ScalarE copy
                ps_r = trps.tile([128, 512], F32, tag="trr")
                ps_n = trps.tile([128, 512], F32, tag="trn")
                for k in range(NK):
                    nc.tensor.transpose(ps_r[:, k * 128:(k + 1) * 128],
                                        xr_t[:, k * 128:(k + 1) * 128], ident)
                    nc.tensor.transpose(ps_n[:, k * 128:(k + 1) * 128],
                                        xn_t[:, k * 128:(k + 1) * 128], ident)
                nc.vector.tensor_copy(
                    xrT[:, :, t * 128:(t + 1) * 128],
                    ps_r.rearrange("p (k i) -> p k i", k=NK))
                nc.scalar.copy(
                    out=xnT[:, :, t * 128:(t + 1) * 128],
                    in_=ps_n.rearrange("p (k i) -> p k i", k=NK))

            def p2_v(t):
                # v (all heads, token-major) for tile t -> ccq v section
                ps = vps.tile([128, D], F32, tag="v")
                for k in range(NK):
                    nc.tensor.matmul(ps, xnT[:, k, t * 128:(t + 1) * 128],
                                     wv_sb[:, k, :],
                                     start=(k == 0), stop=(k == NK - 1))
                stv = stg.tile([128, H, Dh], BF16, tag="stv")
                if has_bv:
                    nc.vector.tensor_add(
                        stv.rearrange("p h d -> p (h d)"), ps, bv_bc)
                else:
                    nc.scalar.copy(out=stv.rearrange("p h d -> p (h d)"), in_=ps)
                nc.sync.dma_start(
                    out=ccq_in[:, :, QKC + t * Dh:QKC + (t + 1) * Dh].rearrange(
                        "w p d -> p w d"),
                    in_=stv)

            def p2_qk(b):
                # q,k head-major for batch b -> ccq qk section
                for h in range(H):
                    ps = qkps.tile([128, SL], F32, tag="qk")
                    for k in range(NK):
                        nc.tensor.matmul(ps, wqk_sb[:, k, h * 128:(h + 1) * 128],
                                         xrT[:, k, b * SL:(b + 1) * SL],
                                         start=(k == 0), stop=(k == NK - 1))
                    stq = stg.tile([128, SL], BF16, tag="stq")
                    if has_bqk:
                        nc.vector.tensor_scalar_add(stq, ps, bqk_sb[:, h:h + 1])
                    else:
                        nc.scalar.copy(out=stq, in_=ps)
                    nc.sync.dma_start(out=ccq_in[h, :, b * SL:(b + 1) * SL], in_=stq)

            for t in range(4):
                p1_tile(t)
            for t in range(4):
                p2_v(t)
            p2_qk(0)
            for t in range(4, NT):
                p1_tile(t)
            for t in range(4, NT):
                p2_v(t)
            p2_qk(1)

        if not skip_cc and max_phase >= 2:
            nc.gpsimd.collective_compute(
                "AllToAll", ALU.bypass,
                ins=[ccq_in.opt()], outs=[ccq_out.opt()],
                replica_groups=[list(range(W))],
            )

        with ExitStack() as actx:
          if max_phase >= 3:
            act = actx.enter_context(tc.tile_pool(name="act", bufs=1))
            qT = act.tile([Dh, B, S], BF16)
            kT = act.tile([Dh, B, S], BF16)
            vS = act.tile([128, B, NS, 65], BF16)
            nc.vector.memset(vS[:, :, :, 64:65], 1.0)
            attnT = act.tile([Dh, B, S], BF16)

            # ---- assembly: head c = my rank's block ----
            for b in range(B):
                nc.sync.dma_start(
                    out=qT[:, b, :].rearrange("p (j s) -> p j s", j=W),
                    in_=ccq_out[:, 0:Dh, b * SL:(b + 1) * SL].rearrange(
                        "j p s -> p j s"))
                nc.sync.dma_start(
                    out=kT[:, b, :].rearrange("p (j s) -> p j s", j=W),
                    in_=ccq_out[:, Dh:2 * Dh, b * SL:(b + 1) * SL].rearrange(
                        "j p s -> p j s"))
            for b in range(B):
                for j in range(W):
                    nc.sync.dma_start(
                        out=vS[:, b, j * 4:(j + 1) * 4, 0:64],
                        in_=ccq_out[j, :, QKC + b * 4 * Dh:QKC + (b + 1) * 4 * Dh]
                            .rearrange("p (sc d) -> p sc d", d=Dh))

            # ============ P4: causal attention (software-pipelined) ============
            if max_phase >= 4:
              with ExitStack() as ctx:
                  expp = ctx.enter_context(tc.tile_pool(name="p4e", bufs=4))
                  nrm = ctx.enter_context(tc.tile_pool(name="p4n", bufs=3))
                  scps = ctx.enter_context(tc.tile_pool(name="p4s", bufs=2, space="PSUM"))
                  atps = ctx.enter_context(tc.tile_pool(name="p4a", bufs=2, space="PSUM"))
                  # flat job list: (b, qb, pair)
                  jobs = [(b, qb, p)
                          for b in range(B) for qb in range(8)
                          for p in range(2 * (qb + 1))]
                  sc_ps = {}
                  pa_cur = {}

                  def emit_sc(job):
                      b, qb, p = job
                      q_rhs = qT[:, b, qb * 512:(qb + 1) * 512]
                      ps = scps.tile([128, 1024], F32, tag="sc", name="sc_ps_t")
                      for i in range(2):
                          kt = p * 2 + i
                          nc.tensor.matmul(ps[:, i * 512:(i + 1) * 512],
                                           kT[:, b, kt * 128:(kt + 1) * 128],
                                           q_rhs, start=True, stop=True)
                      sc_ps[job] = ps

                  def emit_pv(job):
                      b, qb, p = job
                      nkt = 4 * (qb + 1)
                      ps = sc_ps.pop(job)
                      if p == 0:
                          pa_cur[(b, qb)] = atps.tile([65, 512], F32, tag="pa",
                                                      name="pa_t")
                      pa = pa_cur[(b, qb)]
                      ex = expp.tile([128, 1024], BF16, tag="ex", name="ex_t")
                      nc.scalar.activation(out=ex, in_=ps, func=AF.Exp, scale=SCALE)
                      for i in range(2):
                          kt = p * 2 + i
                          jm = kt - (nkt - 4)
                          if jm >= 0:
                              nc.vector.tensor_mul(ex[:, i * 512:(i + 1) * 512],
                                                   ex[:, i * 512:(i + 1) * 512],
                                                   masks[:, jm, :])
                          nc.tensor.matmul(pa, vS[:, b, kt, :],
                                           ex[:, i * 512:(i + 1) * 512],
                                           start=(kt == 0), stop=(kt == nkt - 1))
                      if p == 2 * (qb + 1) - 1:
                          # normalization tail for this (b, qb)
                          pa = pa_cur.pop((b, qb))
                          pa_sb = nrm.tile([65, 512], F32, tag="pasb")
                          nc.vector.tensor_copy(pa_sb, pa)
                          sums = nrm.tile([1, 512], F32, tag="sums")
                          nc.sync.dma_start(out=sums, in_=pa_sb[64:65, :])
                          rcp = nrm.tile([1, 512], F32, tag="rcp")
                          nc.vector.reciprocal(rcp, sums)
                          rcp_bc = nrm.tile([Dh, 512], F32, tag="rbc")
                          nc.gpsimd.partition_broadcast(rcp_bc, rcp)
                          nc.vector.tensor_mul(
                              attnT[:, b, qb * 512:(qb + 1) * 512],
                              pa_sb[0:64, :], rcp_bc)

                  emit_sc(jobs[0])
                  for idx, job in enumerate(jobs):
                      if idx + 1 < len(jobs):
                          emit_sc(jobs[idx + 1])
                      emit_pv(job)
                      # ship + exchange each batch as soon as it completes
                      b, qb, p = job
                      if qb == 7 and p == 2 * (qb + 1) - 1:
                          nc.sync.dma_start(
                              out=cc2_in[b].rearrange("j d i -> d j i"),
                              in_=attnT[:, b, :].rearrange("d (j i) -> d j i", j=W))
                          if not skip_cc and max_phase >= 5:
                              nc.gpsimd.collective_compute(
                                  "AllToAll", ALU.bypass,
                                  ins=[cc2_in[b].opt()], outs=[cc2_out[b].opt()],
                                  replica_groups=[list(range(W))],
                              )

        # ========== P5/P6 ==========
        late_ctx = top.enter_context(ExitStack())
        late = late_ctx.enter_context(tc.tile_pool(name="late", bufs=1))
        out1 = late.tile([128, NT, D], F32)   # post-attention residual stream
        yT = late.tile([128, NK, TL], F32R)    # LN2 output, D-major

        # ========== P5: out_proj + residual + LN2 (+ transpose y) ==========
        if max_phase >= 6:
          with ExitStack() as ctx:
              sb = ctx.enter_context(tc.tile_pool(name="p5", bufs=3))
              small = ctx.enter_context(tc.tile_pool(name="p5s", bufs=4))
              ops = ctx.enter_context(tc.tile_pool(name="p5ps", bufs=2, space="PSUM"))
              trps = ctx.enter_context(tc.tile_pool(name="p5tr", bufs=2, space="PSUM"))
              for t in range(NT):
                  b, sc = t // (NT // B), t % (NT // B)
                  po = ops.tile([128, D], F32, tag="po")
                  for k in range(NK):
                      a_sb = sb.tile([128, 128], BF16, tag="a")
                      nc.sync.dma_start(
                          out=a_sb,
                          in_=cc2_out[b][2 * k:2 * k + 2, :,
                                         sc * 128:(sc + 1) * 128].rearrange(
                                             "e d i -> (e d) i"))
                      nc.tensor.matmul(po, a_sb, wo_sb[:, k, :],
                                       start=(k == 0), stop=(k == NK - 1))
                  s_t = sb.tile([128, D], F32, tag="s")
                  nc.sync.dma_start(out=s_t, in_=src_loc[t * 128:(t + 1) * 128, :])
                  o1 = out1[:, t, :]
                  nc.vector.tensor_add(o1, po, s_t)
                  if has_bo:
                      nc.vector.tensor_add(o1, o1, bo_bc)
                  # LN2 (affine folded into w1_t/b1p host-side)
                  rstd, nm = _layer_norm_stats(nc, small, o1, eps_sb)
                  y_t = sb.tile([128, D], F32, tag="y")
                  nc.vector.tensor_scalar(out=y_t, in0=o1, scalar1=rstd, scalar2=nm,
                                          op0=ALU.mult, op1=ALU.add)
                  ps = trps.tile([128, 512], F32, tag="tr")
                  for k in range(NK):
                      nc.tensor.transpose(ps[:, k * 128:(k + 1) * 128],
                                          y_t[:, k * 128:(k + 1) * 128], ident)
                  nc.scalar.copy(
                      out=yT[:, :, t * 128:(t + 1) * 128],
                      in_=ps.rearrange("p (k i) -> p k i", k=NK))

        # ================= P6: FFN + final residual =================
        if max_phase >= 7:
          with ExitStack() as ctx:
              sb = ctx.enter_context(tc.tile_pool(name="p6", bufs=3))
              hps = ctx.enter_context(tc.tile_pool(name="p6h", bufs=2, space="PSUM"))
              o2ps = ctx.enter_context(tc.tile_pool(name="p6o", bufs=1, space="PSUM"))
              for th in range(2):
                  po2 = [o2ps.tile([128, D], F32, tag=f"po2_{tq}", name=f"po2_{tq}")
                         for tq in range(4)]
                  for m in range(NF):
                      ph = hps.tile([128, 512], F32, tag="ph")
                      for k in range(NK):
                          nc.tensor.matmul(ph, w1_sb[:, k, m * 128:(m + 1) * 128],
                                           yT[:, k, th * 512:(th + 1) * 512],
                                           start=(k == 0), stop=(k == NK - 1))
                      hT = sb.tile([128, 512], F32R, tag="hT")
                      nc.scalar.activation(out=hT, in_=ph,
                                           func=_GELU_OVERRIDE or AF.Gelu,
                                           bias=b1_sb[:, m:m + 1])
                      for tq in range(4):
                          nc.tensor.matmul(po2[tq], hT[:, tq * 128:(tq + 1) * 128],
                                           w2_sb[:, m, :],
                                           start=(m == 0), stop=(m == NF - 1))
                  for tq in range(4):
                      t = th * 4 + tq
                      fin = sb.tile([128, D], F32, tag="fin")
                      nc.vector.tensor_add(fin, po2[tq], out1[:, t, :])
                      if has_b2:
                          nc.vector.tensor_add(fin, fin, b2_bc)
                      nc.sync.dma_start(out=out_loc[t * 128:(t + 1) * 128, :], in_=fin)

        if max_phase < 7:
            with tc.tile_pool(name="dummy", bufs=1) as dp:
                dt_ = dp.tile([128, D], F32)
                nc.vector.memset(dt_, 0.0)
                for i in range(TL // 128):
                    nc.sync.dma_start(out=out_loc[i * 128:(i + 1) * 128, :], in_=dt_)
    nc.compile()
    return nc


def _prep(inputs):
    src = np.asarray(inputs["src"], np.float32)
    cos = np.asarray(inputs["rotary_cos"], np.float32).reshape(S, Dh)
    sin = np.asarray(inputs["rotary_sin"], np.float32).reshape(S, Dh)
    ipw = np.asarray(inputs["in_proj_w"], np.float32)
    ipb = np.asarray(inputs["in_proj_b"], np.float32)
    opw = np.asarray(inputs["out_proj_w"], np.float32)
    opb = np.asarray(inputs["out_proj_b"], np.float32)
    w1 = np.asarray(inputs["w1"], np.float32)
    b1 = np.asarray(inputs["b1"], np.float32)
    w2 = np.asarray(inputs["w2"], np.float32)
    b2 = np.asarray(inputs["b2"], np.float32)
    ln1_w = np.asarray(inputs["ln1_w"], np.float32)
    ln1_b = np.asarray(inputs["ln1_b"], np.float32)
    ln2_w = np.asarray(inputs["ln2_w"], np.float32)
    ln2_b = np.asarray(inputs["ln2_b"], np.float32)

    cos_full = np.tile(cos, (1, H))            # [S, D]
    sin_full = np.tile(sin, (1, H))
    d = np.arange(D)
    jj = d % Dh
    hb = d - jj
    src2 = np.where(jj < 32, hb + 2 * jj + 1, hb + 2 * (jj - 32))
    sign = np.where(jj < 32, -1.0, 1.0).astype(np.float32)
    cosw_full = ln1_w[None, :] * cos_full
    rotw_full = (sign[None, :] * ln1_w[src2][None, :]) * sin_full
    ropeb_full = (ln1_b[None, :] * cos_full
                  + (sign[None, :] * ln1_b[src2][None, :]) * sin_full)

    wq, wk, wv = ipw[0:D], ipw[D:2 * D], ipw[2 * D:3 * D]
    bq, bk, bvv = ipb[0:D], ipb[D:2 * D], ipb[2 * D:3 * D]
    # q,k packed h-major: [wq_h.T | wk_h.T] per head
    wqk_cols = []
    for h in range(H):
        wqk_cols.append(wq[h * Dh:(h + 1) * Dh].T)
        wqk_cols.append(wk[h * Dh:(h + 1) * Dh].T)
    wqk_t = np.ascontiguousarray(np.concatenate(wqk_cols, axis=1))  # [D, 1024]
    bqk_pack = np.zeros((128, H), np.float32)
    for h in range(H):
        bqk_pack[0:Dh, h] = bq[h * Dh:(h + 1) * Dh]
        bqk_pack[Dh:2 * Dh, h] = bk[h * Dh:(h + 1) * Dh]
    wv_t = np.ascontiguousarray(ln1_w[:, None] * wv.T, np.float32)  # [D, 512]
    bv_all = np.ascontiguousarray(ln1_b @ wv.T + bvv, np.float32)
    w1_t = np.ascontiguousarray(ln2_w[:, None] * w1.T, np.float32)   # [D, F]
    b1p = np.ascontiguousarray(ln2_b @ w1.T + b1, np.float32)
    wo_t = np.ascontiguousarray(opw.T)

    flags = (
        bool(np.any(ropeb_full)), bool(np.any(bq) or np.any(bk)),
        bool(np.any(bvv) or np.any(ln1_b)), bool(np.any(opb)), bool(np.any(b2)),
    )

    shared = {
        "wqk_t": wqk_t.astype(ml_dtypes.bfloat16),
        "wv_t": wv_t.astype(ml_dtypes.bfloat16),
        "bqk": bqk_pack,
        "bv": bv_all,
        "wo_t": wo_t.astype(ml_dtypes.bfloat16),
        "bo": opb,
        "w1_t": w1_t,
        "b1p": b1p,
        "w2_t": np.ascontiguousarray(w2.T),
        "b2": b2,
    }
    in_maps = []
    for c in range(W):
        m = dict(shared)
        m["src_loc"] = np.ascontiguousarray(
            src[SL * c:SL * (c + 1)].transpose(1, 0, 2).reshape(TL, D))
        m["cosw"] = np.ascontiguousarray(cosw_full[SL * c:SL * (c + 1)])
        m["rotw"] = np.ascontiguousarray(rotw_full[SL * c:SL * (c + 1)])
        if flags[0]:
            m["ropeb"] = np.ascontiguousarray(ropeb_full[SL * c:SL * (c + 1)])
        in_maps.append(m)
    return in_maps, flags


def _get_nc(flags):
    if flags not in _NC_CACHE:
        _NC_CACHE[flags] = _build_nc(flags)
    return _NC_CACHE[flags]


def kernel(**inputs):
    in_maps, flags = _prep(inputs)
    nc = _get_nc(flags)
    res = run_bass_kernel_spmd(nc, in_maps, core_ids=list(range(W)))
    out = np.empty((S, B, D), np.float32)
    for c in range(W):
        ol = res.results[c]["out_loc"].reshape(B, SL, D)
        out[SL * c:SL * (c + 1)] = ol.transpose(1, 0, 2)
    return out


# revision 22
# speedup vs baseline: 1.2130x; 1.0224x over previous
"""Trainium2 Bass kernel: transformer encoder layer (S=4096,B=2,D=512,H=8,F=2048),
causal attention + RoPE, distributed over 8 NeuronCores.

Sharding (SPMD: one program, per-core data):
  - LN1+RoPE: sequence-parallel (core c owns s in [512c, 512(c+1)), both batches)
  - QKV projections: token-parallel (each core projects q,k,v of ALL heads for
    its own tokens, emitting q/k head-major and v token-major directly)
  - AllToAll(qkv)             [384KB/rank] -> core c gets head c for all tokens
  - causal attention: head-parallel (core c owns head c, full S, both b)
  - AllToAll(attn_head^T)     [512KB/rank, per batch] -> all heads, own tokens
  - out_proj + residual + LN2 + FFN: token-parallel (core c owns its s-slice)
LayerNorm affine params are folded into downstream weights host-side.
Softmax denominators come free from a ones-column appended to V.
All weights preloaded to SBUF up front so their DMA overlaps early compute.
"""
import numpy as np
import ml_dtypes
from contextlib import ExitStack

import concourse.bass as bass
import concourse.tile as tile
from concourse import bacc, mybir
from concourse.bass_utils import run_bass_kernel_spmd
from concourse.masks import make_identity

F32 = mybir.dt.float32
F32R = mybir.dt.float32r
BF16 = mybir.dt.bfloat16
AF = mybir.ActivationFunctionType
ALU = mybir.AluOpType

S, B, D, H, Dh, F = 4096, 2, 512, 8, 64, 2048
W = 8                    # cores
SL = S // W              # 512 s-positions per core
TL = SL * B              # 1024 local tokens
EPS = 1e-5
SCALE = 1.0 / float(np.sqrt(Dh))  # 0.125

NT = TL // 128           # 8 local token tiles
NK = D // 128            # 4 contraction chunks over D
NF = F // 128            # 16 chunks over F
NS = S // 128            # 32 key tiles per batch
QKC = 2 * Dh * H         # 1024 packed q,k output columns (h-major)
CCW = QKC + NT * Dh      # 1536 columns per A2A block

_NC_CACHE = {}
_GELU_OVERRIDE = None  # set to AF.Identity in sim tests (CoreSim lacks Gelu)


def _layer_norm_stats(nc, pool, x_t, eps_sb):
    """Returns (rstd [128,1], negmean_rstd [128,1]) for rows of x_t."""
    stats = pool.tile([128, 6], F32, tag="st")
    nc.vector.bn_stats(out=stats, in_=x_t)
    mv = pool.tile([128, 2], F32, tag="mv")
    nc.vector.bn_aggr(out=mv, in_=stats)
    sd = pool.tile([128, 1], F32, tag="sd")
    nc.scalar.activation(out=sd, in_=mv[:, 1:2], func=AF.Sqrt, bias=eps_sb)
    rstd = pool.tile([128, 1], F32, tag="rs")
    nc.vector.reciprocal(out=rstd, in_=sd)
    nm = pool.tile([128, 1], F32, tag="nm")
    nc.vector.tensor_mul(nm, mv[:, 0:1], rstd)
    nc.vector.tensor_scalar_mul(nm, nm, -1.0)
    return rstd, nm


def _build_nc(flags, n_reps=1):
    """flags = (has_ropeb, has_bqk, has_bv, has_bo, has_b2)

    n_reps > 1 builds a timing variant with the body unrolled n_reps times
    (same I/O, idempotent) so device time can be read off the slope.
    """
    import os as _os
    has_ropeb, has_bqk, has_bv, has_bo, has_b2 = flags
    skip_cc = bool(int(_os.environ.get("K_SKIP_CC", "0")))
    max_phase = int(_os.environ.get("K_MAX_PHASE", "7"))
    nc = bacc.Bacc("TRN2", target_bir_lowering=False, debug=False, num_devices=W)

    # ---- I/O ----
    src_loc = nc.dram_tensor("src_loc", [TL, D], F32, kind="ExternalInput")
    cosw = nc.dram_tensor("cosw", [SL, D], F32, kind="ExternalInput")
    rotw = nc.dram_tensor("rotw", [SL, D], F32, kind="ExternalInput")
    ropeb = nc.dram_tensor("ropeb", [SL, D], F32, kind="ExternalInput") if has_ropeb else None
    wqk_t = nc.dram_tensor("wqk_t", [D, QKC], BF16, kind="ExternalInput")
    wv_t = nc.dram_tensor("wv_t", [D, D], BF16, kind="ExternalInput")
    bqk = nc.dram_tensor("bqk", [128, H], F32, kind="ExternalInput")
    bv = nc.dram_tensor("bv", [D], F32, kind="ExternalInput")
    wo_t = nc.dram_tensor("wo_t", [D, D], BF16, kind="ExternalInput")
    bo = nc.dram_tensor("bo", [D], F32, kind="ExternalInput")
    w1_t = nc.dram_tensor("w1_t", [D, F], F32R, kind="ExternalInput")
    b1p = nc.dram_tensor("b1p", [F], F32, kind="ExternalInput")
    w2_t = nc.dram_tensor("w2_t", [F, D], F32R, kind="ExternalInput")
    b2 = nc.dram_tensor("b2", [D], F32, kind="ExternalInput")
    out_loc = nc.dram_tensor("out_loc", [TL, D], F32, kind="ExternalOutput")

    with tile.TileContext(nc) as tc:
      for _rep in range(n_reps):
       with ExitStack() as top:
        dram = top.enter_context(tc.tile_pool(name="dram", bufs=1, space="DRAM"))
        consts = top.enter_context(tc.tile_pool(name="consts", bufs=1))

        # ---------- constants + all weights up front ----------
        ident = consts.tile([128, 128], F32)
        make_identity(nc, ident)
        # causal diag masks: masks[:, j, q] = 1.0 if q >= k + j*128 else 0.0
        masks = consts.tile([128, 4, 512], BF16)
        for j in range(4):
            nc.gpsimd.memset(masks[:, j, :], 1.0)
            nc.gpsimd.affine_select(
                out=masks[:, j, :], in_=masks[:, j, :],
                compare_op=ALU.is_ge, fill=0.0,
                base=-j * 128, channel_multiplier=-1, pattern=[[1, 512]],
            )
        eps_sb = consts.tile([128, 1], F32)
        nc.vector.memset(eps_sb, EPS)
        wqk_sb = consts.tile([128, NK, QKC], BF16)
        nc.sync.dma_start(out=wqk_sb, in_=wqk_t.rearrange("(k p) m -> p k m", p=128))
        wv_sb = consts.tile([128, NK, D], BF16)
        nc.sync.dma_start(out=wv_sb, in_=wv_t.rearrange("(k p) m -> p k m", p=128))
        wo_sb = consts.tile([128, NK, D], BF16)
        nc.sync.dma_start(out=wo_sb, in_=wo_t.rearrange("(k p) n -> p k n", p=128))
        w1_sb = consts.tile([128, NK, F], F32R)
        nc.sync.dma_start(out=w1_sb, in_=w1_t.rearrange("(k p) n -> p k n", p=128))
        w2_sb = consts.tile([128, NF, D], F32R)
        nc.sync.dma_start(out=w2_sb, in_=w2_t.rearrange("(m p) n -> p m n", p=128))
        b1_sb = consts.tile([128, NF], F32)
        nc.sync.dma_start(out=b1_sb, in_=b1p.rearrange("(m p) -> p m", p=128))
        bqk_sb = consts.tile([128, H], F32)
        if has_bqk:
            nc.sync.dma_start(out=bqk_sb, in_=bqk)
        bv_bc = consts.tile([128, D], F32)
        if has_bv:
            bv_row = consts.tile([1, D], F32)
            nc.sync.dma_start(out=bv_row, in_=bv[None, :])
            nc.gpsimd.partition_broadcast(bv_bc, bv_row)
        bo_bc = consts.tile([128, D], F32)
        if has_bo:
            bo_row = consts.tile([1, D], F32)
            nc.sync.dma_start(out=bo_row, in_=bo[None, :])
            nc.gpsimd.partition_broadcast(bo_bc, bo_row)
        b2_bc = consts.tile([128, D], F32)
        if has_b2:
            b2_row = consts.tile([1, D], F32)
            nc.sync.dma_start(out=b2_row, in_=b2[None, :])
            nc.gpsimd.partition_broadcast(b2_bc, b2_row)

        # collective buffers
        ccq_in = dram.tile([W, 128, CCW], BF16)
        ccq_out = dram.tile([W, 128, CCW], BF16)
        cc2_in = [dram.tile([W, Dh, SL], BF16, name=f"cc2_in_{b}") for b in range(B)]
        cc2_out = [dram.tile([W, Dh, SL], BF16, name=f"cc2_out_{b}") for b in range(B)]

        # ============ P1: LN1 + RoPE + transpose (seq-parallel) ============
        # ============ P2': QKV for own tokens, all heads ============
        with ExitStack() as ctx:
            xt_pool = ctx.enter_context(tc.tile_pool(name="xT", bufs=1))
            # D-major LN1/RoPE outputs (live through P2' only)
            xrT = xt_pool.tile([128, NK, TL], BF16)
            xnT = xt_pool.tile([128, NK, TL], BF16)
            sb = ctx.enter_context(tc.tile_pool(name="p1", bufs=3))
            small = ctx.enter_context(tc.tile_pool(name="p1s", bufs=4))
            trps = ctx.enter_context(tc.tile_pool(name="p1ps", bufs=2, space="PSUM"))
            qkps = ctx.enter_context(tc.tile_pool(name="p2qk", bufs=2, space="PSUM"))
            vps = ctx.enter_context(tc.tile_pool(name="p2v", bufs=2, space="PSUM"))
            stg = ctx.enter_context(tc.tile_pool(name="p2stg", bufs=3))

            def p1_tile(t):
                s_t = sb.tile([128, D], F32, tag="s")
                nc.sync.dma_start(out=s_t, in_=src_loc[t * 128:(t + 1) * 128, :])
                rstd, nm = _layer_norm_stats(nc, small, s_t, eps_sb)
                xn_t = sb.tile([128, D], F32, tag="xn")
                nc.vector.tensor_scalar(
                    out=xn_t, in0=s_t, scalar1=rstd, scalar2=nm,
                    op0=ALU.mult, op1=ALU.add,
                )
                # RoPE (ln1 affine folded into cosw/rotw/ropeb host-side)
                sc = t % (SL // 128)
                cosw_t = sb.tile([128, D], F32, tag="cw")
                nc.sync.dma_start(out=cosw_t, in_=cosw[sc * 128:(sc + 1) * 128, :])
                rotw_t = sb.tile([128, D], F32, tag="rw")
                nc.sync.dma_start(out=rotw_t, in_=rotw[sc * 128:(sc + 1) * 128, :])
                xr_t = sb.tile([128, D], F32, tag="xr")
                rt = sb.tile([128, D], F32, tag="rt")
                xnv = xn_t.rearrange("p (h i two) -> p h i two", h=H, two=2)
                rtv = rt.rearrange("p (h d) -> p h d", h=H)
                rwv = rotw_t.rearrange("p (h d) -> p h d", h=H)
                # rt[:, :, :32] = xn[:, :, 1::2] * rotw[:, :, :32]
                nc.vector.tensor_mul(rtv[:, :, 0:32], xnv[:, :, :, 1], rwv[:, :, 0:32])
                # rt[:, :, 32:] = xn[:, :, 0::2] * rotw[:, :, 32:]
                nc.vector.tensor_mul(rtv[:, :, 32:64], xnv[:, :, :, 0], rwv[:, :, 32:64])
                nc.vector.tensor_mul(xr_t, xn_t, cosw_t)
                nc.vector.tensor_add(xr_t, xr_t, rt)
                if has_ropeb:
                    rb_t = sb.tile([128, D], F32, tag="rb")
                    nc.sync.dma_start(out=rb_t, in_=ropeb[sc * 128:(sc + 1) * 128, :])
                    nc.vector.tensor_add(xr_t, xr_t, rb_t)
                # transpose both to D-major; xr -> DVE copy, xn -> ScalarE copy
                ps_r = trps.tile([128, 512], F32, tag="trr")
                ps_n = trps.tile([128, 512], F32, tag="trn")
                for k in range(NK):
                    nc.tensor.transpose(ps_r[:, k * 128:(k + 1) * 128],
                                        xr_t[:, k * 128:(k + 1) * 128], ident)
                    nc.tensor.transpose(ps_n[:, k * 128:(k + 1) * 128],
                                        xn_t[:, k * 128:(k + 1) * 128], ident)
                nc.vector.tensor_copy(
                    xrT[:, :, t * 128:(t + 1) * 128],
                    ps_r.rearrange("p (k i) -> p k i", k=NK))
                nc.scalar.copy(
                    out=xnT[:, :, t * 128:(t + 1) * 128],
                    in_=ps_n.rearrange("p (k i) -> p k i", k=NK))

            def p2_v(t):
                # v (all heads, token-major) for tile t -> ccq v section
                ps = vps.tile([128, D], F32, tag="v")
                for k in range(NK):
                    nc.tensor.matmul(ps, xnT[:, k, t * 128:(t + 1) * 128],
                                     wv_sb[:, k, :],
                                     start=(k == 0), stop=(k == NK - 1))
                stv = stg.tile([128, H, Dh], BF16, tag="stv")
                if has_bv:
                    nc.vector.tensor_add(
                        stv.rearrange("p h d -> p (h d)"), ps, bv_bc)
                else:
                    nc.scalar.copy(out=stv.rearrange("p h d -> p (h d)"), in_=ps)
                nc.sync.dma_start(
                    out=ccq_in[:, :, QKC + t * Dh:QKC + (t + 1) * Dh].rearrange(
                        "w p d -> p w d"),
                    in_=stv)

            def p2_qk(b):
                # q,k head-major for batch b -> ccq qk section
                for h in range(H):
                    ps = qkps.tile([128, SL], F32, tag="qk")
                    for k in range(NK):
                        nc.tensor.matmul(ps, wqk_sb[:, k, h * 128:(h + 1) * 128],
                                         xrT[:, k, b * SL:(b + 1) * SL],
                                         start=(k == 0), stop=(k == NK - 1))
                    stq = stg.tile([128, SL], BF16, tag="stq")
                    if has_bqk:
                        nc.vector.tensor_scalar_add(stq, ps, bqk_sb[:, h:h + 1])
                    else:
                        nc.scalar.copy(out=stq, in_=ps)
                    nc.sync.dma_start(out=ccq_in[h, :, b * SL:(b + 1) * SL], in_=stq)

            for t in range(4):
                p1_tile(t)
            for t in range(4):
                p2_v(t)
            p2_qk(0)
            for t in range(4, NT):
                p1_tile(t)
            for t in range(4, NT):
                p2_v(t)
            p2_qk(1)

        if not skip_cc and max_phase >= 2:
            nc.gpsimd.collective_compute(
                "AllToAll", ALU.bypass,
                ins=[ccq_in.opt()], outs=[ccq_out.opt()],
                replica_groups=[list(range(W))],
            )

        # ---- P5 resources (shared between attention overlap + tail) ----
        late = top.enter_context(tc.tile_pool(name="late", bufs=1))
        out1 = late.tile([128, NT, D], F32)   # post-attention residual stream
        yT = late.tile([128, NK, TL], F32R)    # LN2 output, D-major
        sb5 = top.enter_context(tc.tile_pool(name="p5", bufs=2))
        small5 = top.enter_context(tc.tile_pool(name="p5s", bufs=4))
        p5ps = top.enter_context(tc.tile_pool(name="p5ps", bufs=2, space="PSUM"))

        mv_all = late.tile([128, NT, 2], F32)  # LN2 mean/var per tile

        def p5a_tile(t):
            # out_proj + residual + LN2 stats (PE/DVE only — safe to overlap
            # attention without touching ScalarE's loaded exp table set)
            b, sc = t // (NT // B), t % (NT // B)
            po = p5ps.tile([128, D], F32, tag="p5")
            for k in range(NK):
                a_sb = sb5.tile([128, 128], BF16, tag="a")
                nc.sync.dma_start(
                    out=a_sb,
                    in_=cc2_out[b][2 * k:2 * k + 2, :,
                                   sc * 128:(sc + 1) * 128].rearrange(
                                       "e d i -> (e d) i"))
                nc.tensor.matmul(po, a_sb, wo_sb[:, k, :],
                                 start=(k == 0), stop=(k == NK - 1))
            s_t = sb5.tile([128, D], F32, tag="s")
            nc.sync.dma_start(out=s_t, in_=src_loc[t * 128:(t + 1) * 128, :])
            o1 = out1[:, t, :]
            nc.vector.tensor_add(o1, po, s_t)
            if has_bo:
                nc.vector.tensor_add(o1, o1, bo_bc)
            stats = small5.tile([128, 6], F32, tag="st")
            nc.vector.bn_stats(out=stats, in_=o1)
            nc.vector.bn_aggr(out=mv_all[:, t, :], in_=stats)

        def p5b_tile(t):
            # LN2 normalize + yT transpose (ScalarE sqrt grouped in the tail)
            sd = small5.tile([128, 1], F32, tag="sd")
            nc.scalar.activation(out=sd, in_=mv_all[:, t, 1:2], func=AF.Sqrt,
                                 bias=eps_sb)
            rstd = small5.tile([128, 1], F32, tag="rs")
            nc.vector.reciprocal(out=rstd, in_=sd)
            nm = small5.tile([128, 1], F32, tag="nm")
            nc.vector.tensor_mul(nm, mv_all[:, t, 0:1], rstd)
            nc.vector.tensor_scalar_mul(nm, nm, -1.0)
            y_t = sb5.tile([128, D], F32, tag="y")
            nc.vector.tensor_scalar(out=y_t, in0=out1[:, t, :], scalar1=rstd,
                                    scalar2=nm, op0=ALU.mult, op1=ALU.add)
            ps = p5ps.tile([128, 512], F32, tag="p5")
            for k in range(NK):
                nc.tensor.transpose(ps[:, k * 128:(k + 1) * 128],
                                    y_t[:, k * 128:(k + 1) * 128], ident)
            nc.scalar.copy(
                out=yT[:, :, t * 128:(t + 1) * 128],
                in_=ps.rearrange("p (k i) -> p k i", k=NK))

        with ExitStack() as actx:
          if max_phase >= 3:
            act = actx.enter_context(tc.tile_pool(name="act", bufs=1))
            qT = act.tile([Dh, B, S], BF16)
            kT = act.tile([Dh, B, S], BF16)
            vS = act.tile([128, B, NS, 65], BF16)
            nc.vector.memset(vS[:, :, :, 64:65], 1.0)
            attnT = act.tile([Dh, B, S], BF16)

            # ---- assembly: head c = my rank's block ----
            for b in range(B):
                nc.sync.dma_start(
                    out=qT[:, b, :].rearrange("p (j s) -> p j s", j=W),
                    in_=ccq_out[:, 0:Dh, b * SL:(b + 1) * SL].rearrange(
                        "j p s -> p j s"))
                nc.sync.dma_start(
                    out=kT[:, b, :].rearrange("p (j s) -> p j s", j=W),
                    in_=ccq_out[:, Dh:2 * Dh, b * SL:(b + 1) * SL].rearrange(
                        "j p s -> p j s"))
            for b in range(B):
                for j in range(W):
                    nc.sync.dma_start(
                        out=vS[:, b, j * 4:(j + 1) * 4, 0:64],
                        in_=ccq_out[j, :, QKC + b * 4 * Dh:QKC + (b + 1) * 4 * Dh]
                            .rearrange("p (sc d) -> p sc d", d=Dh))

            # ============ P4: causal attention (software-pipelined) ============
            if max_phase >= 4:
              with ExitStack() as ctx:
                  expp = ctx.enter_context(tc.tile_pool(name="p4e", bufs=3))
                  nrm = ctx.enter_context(tc.tile_pool(name="p4n", bufs=2))
                  scps = ctx.enter_context(tc.tile_pool(name="p4s", bufs=2, space="PSUM"))
                  atps = ctx.enter_context(tc.tile_pool(name="p4a", bufs=2, space="PSUM"))
                  # flat job list: (b, qb, pair)
                  jobs = [(b, qb, p)
                          for b in range(B) for qb in range(8)
                          for p in range(2 * (qb + 1))]
                  sc_ps = {}
                  pa_cur = {}

                  def emit_sc(job):
                      b, qb, p = job
                      q_rhs = qT[:, b, qb * 512:(qb + 1) * 512]
                      ps = scps.tile([128, 1024], F32, tag="sc", name="sc_ps_t")
                      for i in range(2):
                          kt = p * 2 + i
                          nc.tensor.matmul(ps[:, i * 512:(i + 1) * 512],
                                           kT[:, b, kt * 128:(kt + 1) * 128],
                                           q_rhs, start=True, stop=True)
                      sc_ps[job] = ps

                  def emit_pv(job):
                      b, qb, p = job
                      nkt = 4 * (qb + 1)
                      ps = sc_ps.pop(job)
                      if p == 0:
                          pa_cur[(b, qb)] = atps.tile([65, 512], F32, tag="pa",
                                                      name="pa_t")
                      pa = pa_cur[(b, qb)]
                      ex = expp.tile([128, 1024], BF16, tag="ex", name="ex_t")
                      nc.scalar.activation(out=ex, in_=ps, func=AF.Exp, scale=SCALE)
                      for i in range(2):
                          kt = p * 2 + i
                          jm = kt - (nkt - 4)
                          if jm >= 0:
                              nc.vector.tensor_mul(ex[:, i * 512:(i + 1) * 512],
                                                   ex[:, i * 512:(i + 1) * 512],
                                                   masks[:, jm, :])
                          nc.tensor.matmul(pa, vS[:, b, kt, :],
                                           ex[:, i * 512:(i + 1) * 512],
                                           start=(kt == 0), stop=(kt == nkt - 1))
                      if p == 2 * (qb + 1) - 1:
                          # normalization tail for this (b, qb)
                          pa = pa_cur.pop((b, qb))
                          pa_sb = nrm.tile([65, 512], F32, tag="pasb")
                          nc.vector.tensor_copy(pa_sb, pa)
                          sums = nrm.tile([1, 512], F32, tag="sums")
                          nc.sync.dma_start(out=sums, in_=pa_sb[64:65, :])
                          nc.vector.reciprocal(sums, sums)
                          rcp_bc = nrm.tile([Dh, 512], F32, tag="rbc")
                          nc.gpsimd.partition_broadcast(rcp_bc, sums)
                          nc.vector.tensor_mul(
                              attnT[:, b, qb * 512:(qb + 1) * 512],
                              pa_sb[0:64, :], rcp_bc)

                  # P5 for b0 token tiles overlaps b1's attention (deps via
                  # cc2_out[0], ready once the b0 AllToAll lands)
                  p5_overlap = {83: 0, 95: 1, 107: 2, 119: 3} if max_phase >= 6 \
                      else {}
                  emit_sc(jobs[0])
                  for idx, job in enumerate(jobs):
                      if idx + 1 < len(jobs):
                          emit_sc(jobs[idx + 1])
                      emit_pv(job)
                      # ship + exchange each batch as soon as it completes
                      b, qb, p = job
                      if qb == 7 and p == 2 * (qb + 1) - 1:
                          nc.sync.dma_start(
                              out=cc2_in[b].rearrange("j d i -> d j i"),
                              in_=attnT[:, b, :].rearrange("d (j i) -> d j i", j=W))
                          if not skip_cc and max_phase >= 5:
                              nc.gpsimd.collective_compute(
                                  "AllToAll", ALU.bypass,
                                  ins=[cc2_in[b].opt()], outs=[cc2_out[b].opt()],
                                  replica_groups=[list(range(W))],
                              )
                      if idx in p5_overlap:
                          p5a_tile(p5_overlap[idx])

        # ========== tail: P6(th0) -> P5(b1) -> P6(th1) ==========
        # P6 th0 depends only on b0's yT (done during attention), so it runs
        # on PE while the b1 AllToAll completes in the background.
        if max_phase >= 7:
          with ExitStack() as ctx:
              sb = ctx.enter_context(tc.tile_pool(name="p6", bufs=3))
              hps = ctx.enter_context(tc.tile_pool(name="p6h", bufs=2, space="PSUM"))
              o2ps = ctx.enter_context(tc.tile_pool(name="p6o", bufs=1, space="PSUM"))

              def p6_half(th):
                  po2 = [o2ps.tile([128, D], F32, tag=f"po2_{tq}", name=f"po2_{tq}")
                         for tq in range(4)]
                  for m in range(NF):
                      ph = hps.tile([128, 512], F32, tag="ph")
                      for k in range(NK):
                          nc.tensor.matmul(ph, w1_sb[:, k, m * 128:(m + 1) * 128],
                                           yT[:, k, th * 512:(th + 1) * 512],
                                           start=(k == 0), stop=(k == NK - 1))
                      hT = sb.tile([128, 512], F32R, tag="hT")
                      nc.scalar.activation(out=hT, in_=ph,
                                           func=_GELU_OVERRIDE or AF.Gelu,
                                           bias=b1_sb[:, m:m + 1])
                      for tq in range(4):
                          nc.tensor.matmul(po2[tq], hT[:, tq * 128:(tq + 1) * 128],
                                           w2_sb[:, m, :],
                                           start=(m == 0), stop=(m == NF - 1))
                  for tq in range(4):
                      t = th * 4 + tq
                      fin = sb.tile([128, D], F32, tag="fin")
                      nc.vector.tensor_add(fin, po2[tq], out1[:, t, :])
                      if has_b2:
                          nc.vector.tensor_add(fin, fin, b2_bc)
                      nc.sync.dma_start(out=out_loc[t * 128:(t + 1) * 128, :], in_=fin)

              for t in range(4):
                  p5b_tile(t)
              p6_half(0)
              for t in range(4, NT):
                  p5a_tile(t)
              for t in range(4, NT):
                  p5b_tile(t)
              p6_half(1)

        if max_phase < 7:
            with tc.tile_pool(name="dummy", bufs=1) as dp:
                dt_ = dp.tile([128, D], F32)
                nc.vector.memset(dt_, 0.0)
                for i in range(TL // 128):
                    nc.sync.dma_start(out=out_loc[i * 128:(i + 1) * 128, :], in_=dt_)
    nc.compile()
    return nc


def _prep(inputs):
    src = np.asarray(inputs["src"], np.float32)
    cos = np.asarray(inputs["rotary_cos"], np.float32).reshape(S, Dh)
    sin = np.asarray(inputs["rotary_sin"], np.float32).reshape(S, Dh)
    ipw = np.asarray(inputs["in_proj_w"], np.float32)
    ipb = np.asarray(inputs["in_proj_b"], np.float32)
    opw = np.asarray(inputs["out_proj_w"], np.float32)
    opb = np.asarray(inputs["out_proj_b"], np.float32)
    w1 = np.asarray(inputs["w1"], np.float32)
    b1 = np.asarray(inputs["b1"], np.float32)
    w2 = np.asarray(inputs["w2"], np.float32)
    b2 = np.asarray(inputs["b2"], np.float32)
    ln1_w = np.asarray(inputs["ln1_w"], np.float32)
    ln1_b = np.asarray(inputs["ln1_b"], np.float32)
    ln2_w = np.asarray(inputs["ln2_w"], np.float32)
    ln2_b = np.asarray(inputs["ln2_b"], np.float32)

    cos_full = np.tile(cos, (1, H))            # [S, D]
    sin_full = np.tile(sin, (1, H))
    d = np.arange(D)
    jj = d % Dh
    hb = d - jj
    src2 = np.where(jj < 32, hb + 2 * jj + 1, hb + 2 * (jj - 32))
    sign = np.where(jj < 32, -1.0, 1.0).astype(np.float32)
    cosw_full = ln1_w[None, :] * cos_full
    rotw_full = (sign[None, :] * ln1_w[src2][None, :]) * sin_full
    ropeb_full = (ln1_b[None, :] * cos_full
                  + (sign[None, :] * ln1_b[src2][None, :]) * sin_full)

    wq, wk, wv = ipw[0:D], ipw[D:2 * D], ipw[2 * D:3 * D]
    bq, bk, bvv = ipb[0:D], ipb[D:2 * D], ipb[2 * D:3 * D]
    # q,k packed h-major: [wq_h.T | wk_h.T] per head
    wqk_cols = []
    for h in range(H):
        wqk_cols.append(wq[h * Dh:(h + 1) * Dh].T)
        wqk_cols.append(wk[h * Dh:(h + 1) * Dh].T)
    wqk_t = np.ascontiguousarray(np.concatenate(wqk_cols, axis=1))  # [D, 1024]
    bqk_pack = np.zeros((128, H), np.float32)
    for h in range(H):
        bqk_pack[0:Dh, h] = bq[h * Dh:(h + 1) * Dh]
        bqk_pack[Dh:2 * Dh, h] = bk[h * Dh:(h + 1) * Dh]
    wv_t = np.ascontiguousarray(ln1_w[:, None] * wv.T, np.float32)  # [D, 512]
    bv_all = np.ascontiguousarray(ln1_b @ wv.T + bvv, np.float32)
    w1_t = np.ascontiguousarray(ln2_w[:, None] * w1.T, np.float32)   # [D, F]
    b1p = np.ascontiguousarray(ln2_b @ w1.T + b1, np.float32)
    wo_t = np.ascontiguousarray(opw.T)

    flags = (
        bool(np.any(ropeb_full)), bool(np.any(bq) or np.any(bk)),
        bool(np.any(bvv) or np.any(ln1_b)), bool(np.any(opb)), bool(np.any(b2)),
    )

    shared = {
        "wqk_t": wqk_t.astype(ml_dtypes.bfloat16),
        "wv_t": wv_t.astype(ml_dtypes.bfloat16),
        "bqk": bqk_pack,
        "bv": bv_all,
        "wo_t": wo_t.astype(ml_dtypes.bfloat16),
        "bo": opb,
        "w1_t": w1_t,
        "b1p": b1p,
        "w2_t": np.ascontiguousarray(w2.T),
        "b2": b2,
    }
    in_maps = []
    for c in range(W):
        m = dict(shared)
        m["src_loc"] = np.ascontiguousarray(
            src[SL * c:SL * (c + 1)].transpose(1, 0, 2).reshape(TL, D))
        m["cosw"] = np.ascontiguousarray(cosw_full[SL * c:SL * (c + 1)])
        m["rotw"] = np.ascontiguousarray(rotw_full[SL * c:SL * (c + 1)])
        if flags[0]:
            m["ropeb"] = np.ascontiguousarray(ropeb_full[SL * c:SL * (c + 1)])
        in_maps.append(m)
    return in_maps, flags


def _get_nc(flags):
    if flags not in _NC_CACHE:
        _NC_CACHE[flags] = _build_nc(flags)
    return _NC_CACHE[flags]


def kernel(**inputs):
    in_maps, flags = _prep(inputs)
    nc = _get_nc(flags)
    res = run_bass_kernel_spmd(nc, in_maps, core_ids=list(range(W)))
    out = np.empty((S, B, D), np.float32)
    for c in range(W):
        ol = res.results[c]["out_loc"].reshape(B, SL, D)
        out[SL * c:SL * (c + 1)] = ol.transpose(1, 0, 2)
    return out
